# revision 15
# baseline (speedup 1.0000x reference)
"""Trainium2 Bass kernel for nn_ChannelSpatialAttention2 (dense_cnn).

Data-parallel over batch: 16 samples / 8 cores = 2 samples per core, no
cross-core communication.  All HBM I/O in bf16 (host casts fp32<->bf16).

Per-sample dataflow (channel-major layout (128 ch, 16384 px), px = h*128+w):

  1. fv/fi loaded full (bf16).  Pixel sums: sv via DVE halving add-trees
     (bf16 tensor_tensor runs in 2x mode; reduces are 1x so trees win),
     si via scalar-engine copies with accum_out.
  2. Tiny MLP (1x1 convs + BN folded on host) -> channel weights a.
  3. conv1 g = lv@fv + li@fi with per-sample stationaries (BN scale
     folded into W on host), 4 matmuls per LDWEIGHTS, [C,1024] psum
     tiles; evac via scalar ReLU+bias with avg-pool riding accum_out.
     Pixel-max per channel via DVE max-trees per quarter.
  4. Channel-mean/max maps: per-quarter DMA xbar transpose g -> gt
     [w, h, c]; DVE add-tree (into scratch, gt preserved) and max-tree
     (in place) over c -> [w, h] pads for the banded conv.
  5. 7x7 spatial conv = 14 accumulating matmuls vs host-built banded
     matrices; sigmoid -> sa [h, w]; flatten -> sa_fl [1, P].
  6. t = sigmoid(ca x sa) via K=1 PE matmuls + scalar sigmoid evac.
  7. blend out = u*fv + (apl1-u)*fi, u = oma*t + a:
     u (DVE ts 4x, in place on t), vv = apl1 - u (DVE ts 4x),
     q2 = vv*fi (gpsimd TT, in place on vv), q1 = u*fv (DVE, in place
     on t), q1 += q2 (DVE), DMA out bf16.
"""

import sys

if '/opt/trn_rl_repo' not in sys.path:
    sys.path.insert(0, '/opt/trn_rl_repo')

import numpy as np
import ml_dtypes

import concourse.bacc as bacc
import concourse.mybir as mybir
import concourse.tile as tile
import concourse.bass_utils as bass_utils

EPS = 1e-5
C = 128
N, H, W = 16, 128, 128
P = H * W            # 16384 pixels per sample
NCORES = 8
SPC = N // NCORES    # samples per core

BF16 = mybir.dt.bfloat16
F32 = mybir.dt.float32
AL = mybir.AluOpType
AF = mybir.ActivationFunctionType
AX = mybir.AxisListType

_cache = {}

QP = 4096            # quarter size (map/blend granularity)


def _build_program():
    nc = bacc.Bacc("TRN2", target_bir_lowering=False, debug=False,
                   enable_asserts=False, num_devices=NCORES)

    d_fvi = nc.dram_tensor("f_vi", (SPC, C, P), BF16, kind="ExternalInput").ap()
    d_fir = nc.dram_tensor("f_ir", (SPC, C, P), BF16, kind="ExternalInput").ap()
    d_out = nc.dram_tensor("out", (SPC, C, P), BF16, kind="ExternalOutput").ap()

    d_wvi = nc.dram_tensor("wvi_t", (C, C), BF16, kind="ExternalInput").ap()
    d_wir = nc.dram_tensor("wir_t", (C, C), BF16, kind="ExternalInput").ap()
    d_bmat = nc.dram_tensor("bmat", (14, 128, 128), BF16, kind="ExternalInput").ap()
    d_l1v = nc.dram_tensor("l1v", (C, C // 2), F32, kind="ExternalInput").ap()
    d_l1i = nc.dram_tensor("l1i", (C, C // 2), F32, kind="ExternalInput").ap()
    d_b1 = nc.dram_tensor("b1", (C // 2, 1), F32, kind="ExternalInput").ap()
    d_l2 = nc.dram_tensor("l2", (C // 2, C), F32, kind="ExternalInput").ap()
    d_b2 = nc.dram_tensor("b2", (C, 1), F32, kind="ExternalInput").ap()
    d_bc = nc.dram_tensor("bc", (C, 1), F32, kind="ExternalInput").ap()
    d_c1a = nc.dram_tensor("c1a", (C, 8), F32, kind="ExternalInput").ap()
    d_c1m = nc.dram_tensor("c1m", (C, 8), F32, kind="ExternalInput").ap()
    d_c2r = nc.dram_tensor("c2r", (8, C), F32, kind="ExternalInput").ap()
    d_eye = nc.dram_tensor("eye", (C, C), BF16, kind="ExternalInput").ap()
    d_sasc = nc.dram_tensor("sa_scr", (SPC, 1, P), BF16, kind="Internal").ap()

    with tile.TileContext(nc) as tc:
        with (
            tc.tile_pool(name="wts", bufs=1) as wts,
            tc.tile_pool(name="fv", bufs=2) as fvp,
            tc.tile_pool(name="fi", bufs=2) as fip,
            tc.tile_pool(name="g", bufs=2) as gp,
            tc.tile_pool(name="gt", bufs=1) as gtp,
            tc.tile_pool(name="t", bufs=2) as tp,
            tc.tile_pool(name="vv", bufs=2) as vvp,
            tc.tile_pool(name="tr", bufs=2) as trp,
            tc.tile_pool(name="sm", bufs=2) as sm,
            tc.tile_pool(name="fl", bufs=2) as flp,
            tc.tile_pool(name="ps", bufs=4, space="PSUM") as ps,
        ):
            # ---- constant loads ----
            wvi = wts.tile([C, C], BF16)
            wir = wts.tile([C, C], BF16)
            bmat = wts.tile([128, 14, 128], BF16)
            eye = wts.tile([C, C], BF16)
            nc.sync.dma_start(wvi[:], d_wvi[:])
            nc.sync.dma_start(wir[:], d_wir[:])
            nc.sync.dma_start(eye[:], d_eye[:])
            nc.sync.dma_start(bmat[:], d_bmat.rearrange("m r c -> r m c"))
            l1v = wts.tile([C, C // 2], F32)
            l1i = wts.tile([C, C // 2], F32)
            b1 = wts.tile([C // 2, 1], F32)
            l2 = wts.tile([C // 2, C], F32)
            b2 = wts.tile([C, 1], F32)
            bc = wts.tile([C, 1], F32)
            c1a = wts.tile([C, 8], F32)
            c1m = wts.tile([C, 8], F32)
            c2r = wts.tile([8, C], F32)
            for t_, dd in ((l1v, d_l1v), (l1i, d_l1i), (b1, d_b1), (l2, d_l2),
                           (b2, d_b2), (bc, d_bc), (c1a, d_c1a),
                           (c1m, d_c1m), (c2r, d_c2r)):
                nc.sync.dma_start(t_[:], dd[:])

            def tree_col(eng, scr, x_q, col, op, lo):
                """Halving op-tree over x_q [C, 4096] -> col [C, 1].

                First fold writes scr[:, :2048] (x_q preserved), then in
                place on scr down to width `lo`, then a 1x reduce.
                """
                nc.vector.tensor_tensor(out=scr[:, 0:2048], in0=x_q[:, 0:2048],
                                        in1=x_q[:, 2048:4096], op=op)
                w_ = 1024
                while w_ >= lo:
                    nc.vector.tensor_tensor(out=scr[:, 0:w_], in0=scr[:, 0:w_],
                                            in1=scr[:, w_:2 * w_], op=op)
                    w_ //= 2
                nc.vector.tensor_reduce(col, scr[:, 0:lo], axis=AX.X, op=op)

            for s in range(SPC):
                # ---- loads ----
                fv_t = fvp.tile([C, P], BF16, tag="fv", name=f"fv{s}")
                fi_t = fip.tile([C, P], BF16, tag="fi", name=f"fi{s}")
                for q in range(4):
                    ql = slice(q * QP, (q + 1) * QP)
                    nc.sync.dma_start(fv_t[:, ql], d_fvi[s][:, ql])
                    nc.sync.dma_start(fi_t[:, ql], d_fir[s][:, ql])

                # ---- pixel sums: sv via DVE trees, si via ACT copy+accum ----
                svp = sm.tile([C, 8], F32, tag="svp", name=f"svp{s}")
                sip = sm.tile([C, 8], F32, tag="sip", name=f"sip{s}")
                for q in range(4):
                    ql = slice(q * QP, (q + 1) * QP)
                    scr = trp.tile([C, 2048], BF16, tag="tr", name=f"svt{s}_{q}")
                    with nc.allow_low_precision(reason="bf16 tree; tol 2e-2"):
                        tree_col(nc.vector, scr, fv_t[:, ql],
                                 svp[:, q:q + 1], AL.add, 128)
                    junk = tp.tile([C, QP], BF16, tag="t", name=f"sij{s}_{q}")
                    nc.scalar.activation(junk[:], fi_t[:, ql], AF.Copy,
                                         accum_out=sip[:, q:q + 1])
                sv = sm.tile([C, 1], F32, tag="sv", name=f"sv{s}")
                si = sm.tile([C, 1], F32, tag="si", name=f"si{s}")
                nc.vector.reduce_sum(sv[:], svp[:, 0:4], axis=AX.X)
                nc.vector.reduce_sum(si[:], sip[:, 0:4], axis=AX.X)

                # ---- channel-avg-attention MLP -> a ----
                ps1 = ps.tile([C, 1024], F32, tag="ps", name=f"ps1_{s}")
                nc.tensor.matmul(ps1[0:64, 0:1], l1v[:], sv[:], start=True, stop=False)
                nc.tensor.matmul(ps1[0:64, 0:1], l1i[:], si[:], start=False, stop=True)
                h1 = sm.tile([C // 2, 1], F32, tag="h1", name=f"h1_{s}")
                nc.scalar.activation(h1[:], ps1[0:64, 0:1], AF.Relu, bias=b1[:])
                ps2 = ps.tile([C, 1024], F32, tag="ps", name=f"ps2_{s}")
                nc.tensor.matmul(ps2[0:C, 0:1], l2[:], h1[:], start=True, stop=True)
                a_col = sm.tile([C, 1], F32, tag="a_col", name=f"a_col{s}")
                nc.scalar.activation(a_col[:], ps2[0:C, 0:1], AF.Sigmoid, bias=b2[:])
                oma = sm.tile([C, 1], F32, tag="oma", name=f"oma{s}")
                nc.vector.tensor_scalar(oma[:], a_col[:], -1.0, 1.0, AL.mult, AL.add)
                apl1 = sm.tile([C, 1], F32, tag="apl1", name=f"apl1{s}")
                nc.vector.tensor_scalar(apl1[:], a_col[:], 1.0, 1.0, AL.mult, AL.add)

                # per-sample effective weights
                lv = sm.tile([C, C], BF16, tag="lv", name=f"lv{s}")
                li = sm.tile([C, C], BF16, tag="li", name=f"li{s}")
                nc.vector.scalar_tensor_tensor(lv[:], wir[:], a_col[:], wvi[:],
                                               AL.mult, AL.add)
                nc.vector.scalar_tensor_tensor(li[:], wvi[:], a_col[:], wir[:],
                                               AL.mult, AL.add)

                # ---- conv1 + evac; per-quarter stats/maps ----
                avp = sm.tile([C, 16], F32, tag="avp", name=f"avp{s}")
                mx8 = sm.tile([C, 8], F32, tag="mx8", name=f"mx8{s}")
                sumpad = sm.tile([128, 134], BF16, tag="sumpad", name=f"sumpad{s}")
                maxpad = sm.tile([128, 134], BF16, tag="maxpad", name=f"maxpad{s}")
                for pad in (sumpad, maxpad):
                    nc.vector.memset(pad[:, 0:3], 0.0)
                    nc.vector.memset(pad[:, 131:134], 0.0)
                for q in range(4):
                    g_q = gp.tile([C, QP], BF16, tag="g", name=f"g{s}_{q}")
                    for h2 in range(4):
                        j = q * 4 + h2
                        pg = ps.tile([C, 1024], F32, tag="ps", name=f"pg{s}_{j}")
                        for b in range(2):
                            sl = slice(j * 1024 + b * 512, j * 1024 + (b + 1) * 512)
                            nc.tensor.matmul(pg[:, b * 512:(b + 1) * 512],
                                             lv[:], fv_t[:, sl],
                                             start=True, stop=False)
                        for b in range(2):
                            sl = slice(j * 1024 + b * 512, j * 1024 + (b + 1) * 512)
                            nc.tensor.matmul(pg[:, b * 512:(b + 1) * 512],
                                             li[:], fi_t[:, sl],
                                             start=False, stop=True)
                        nc.scalar.activation(g_q[:, h2 * 1024:(h2 + 1) * 1024],
                                             pg[:], AF.Relu, bias=bc[:],
                                             accum_out=avp[:, j:j + 1])
                    # pixel-max tree for this quarter
                    scr = trp.tile([C, 2048], BF16, tag="tr", name=f"gmx{s}_{q}")
                    tree_col(nc.vector, scr, g_q[:], mx8[:, q:q + 1], AL.max, 128)
                    # transpose -> gt [w, h, c]; channel maps
                    gt = gtp.tile([128, QP], BF16, tag="gt", name=f"gt{s}_{q}")
                    gt3 = gt[:].rearrange("p (h c) -> p h c", c=128)
                    nc.sync.dma_start_transpose(gt3, g_q[:])
                    # channel-sum tree (gt preserved) -> sumpad cols
                    mscr = trp.tile([C, 2048], BF16, tag="tr", name=f"cs{s}_{q}")
                    ms3 = mscr[:].rearrange("p (h c) -> p h c", c=64)
                    with nc.allow_low_precision(reason="map feeds sigmoid"):
                        nc.vector.tensor_tensor(out=ms3[:], in0=gt3[:, :, 0:64],
                                                in1=gt3[:, :, 64:128], op=AL.add)
                        w_ = 32
                        while w_ >= 1:
                            nc.vector.tensor_tensor(
                                out=ms3[:, :, 0:w_], in0=ms3[:, :, 0:w_],
                                in1=ms3[:, :, w_:2 * w_], op=AL.add)
                            w_ //= 2
                        nc.vector.tensor_copy(
                            sumpad[:, 3 + q * 32:3 + q * 32 + 32]
                            .rearrange("p (f o) -> p f o", o=1),
                            ms3[:, :, 0:1])
                    # channel-max tree (in place on gt) -> maxpad cols
                    w_ = 64
                    while w_ >= 1:
                        nc.vector.tensor_tensor(
                            out=gt3[:, :, 0:w_], in0=gt3[:, :, 0:w_],
                            in1=gt3[:, :, w_:2 * w_], op=AL.max)
                        w_ //= 2
                    nc.vector.tensor_copy(
                        maxpad[:, 3 + q * 32:3 + q * 32 + 32]
                        .rearrange("p (f o) -> p f o", o=1),
                        gt3[:, :, 0:1])

                avs = sm.tile([C, 1], F32, tag="avs", name=f"avs{s}")
                nc.vector.reduce_sum(avs[:], avp[:], axis=AX.X)
                mx = sm.tile([C, 1], F32, tag="mx", name=f"mx{s}")
                nc.vector.reduce_max(mx[:], mx8[:, 0:4], axis=AX.X)

                # ---- ChannelAttention MLP -> ca row ----
                psa = ps.tile([C, 1024], F32, tag="ps", name=f"psa{s}")
                nc.tensor.matmul(psa[0:8, 0:1], c1a[:], avs[:], start=True, stop=True)
                nc.tensor.matmul(psa[0:8, 512:513], c1m[:], mx[:], start=True, stop=True)
                ha = sm.tile([8, 2], F32, tag="ha", name=f"ha{s}")
                nc.scalar.activation(ha[:, 0:1], psa[0:8, 0:1], AF.Relu)
                nc.scalar.activation(ha[:, 1:2], psa[0:8, 512:513], AF.Relu)
                psr = ps.tile([C, 1024], F32, tag="ps", name=f"psr{s}")
                nc.tensor.matmul(psr[0:1, 0:C], ha[:, 0:1], c2r[:],
                                 start=True, stop=False)
                nc.tensor.matmul(psr[0:1, 0:C], ha[:, 1:2], c2r[:],
                                 start=False, stop=True)
                ca = sm.tile([1, C], BF16, tag="ca", name=f"ca{s}")
                nc.scalar.activation(ca[:], psr[0:1, 0:C], AF.Sigmoid)

                # ---- SpatialAttention: 7x7 conv as banded matmuls ----
                pss = ps.tile([C, 1024], F32, tag="ps", name=f"pss{s}")
                first = True
                for chn, pad in ((0, sumpad), (1, maxpad)):
                    for dy in range(7):
                        nc.tensor.matmul(pss[0:128, 0:128], pad[:, dy:dy + 128],
                                         bmat[:, chn * 7 + dy, :],
                                         start=first, stop=(chn == 1 and dy == 6))
                        first = False
                sa_hw = sm.tile([128, 128], BF16, tag="sa_hw", name=f"sa_hw{s}")
                nc.scalar.activation(sa_hw[:], pss[0:128, 0:128], AF.Sigmoid)
                # flatten via HBM round-trip (a [1, P] SBUF tile would
                # reserve a full 32KB column across all partitions)
                nc.sync.dma_start(d_sasc[s][:], sa_hw[:])

                # ---- t field + blend per quarter ----
                for q in range(4):
                    ql = slice(q * QP, (q + 1) * QP)
                    t_q = tp.tile([C, QP], BF16, tag="t", name=f"t{s}_{q}")
                    for h2 in range(4):
                        j = q * 4 + h2
                        sa_c = flp.tile([1, 1024], BF16, tag="sa",
                                        name=f"sa{s}_{j}")
                        nc.sync.dma_start(sa_c[:],
                                          d_sasc[s][:, j * 1024:(j + 1) * 1024])
                        pz = ps.tile([C, 1024], F32, tag="ps", name=f"pz{s}_{j}")
                        for b in range(2):
                            nc.tensor.matmul(pz[:, b * 512:(b + 1) * 512],
                                             ca[:], sa_c[:, b * 512:(b + 1) * 512],
                                             start=True, stop=True)
                        nc.scalar.activation(t_q[:, h2 * 1024:(h2 + 1) * 1024],
                                             pz[:], AF.Sigmoid)
                    # u = oma*t + a (in place on t_q);  vv = apl1 - u
                    nc.vector.tensor_scalar(t_q[:], t_q[:], oma[:], a_col[:],
                                            AL.mult, AL.add)
                    vv = vvp.tile([C, QP], BF16, tag="vv", name=f"vv{s}_{q}")
                    nc.vector.tensor_scalar(vv[:], t_q[:], -1.0, apl1[:],
                                            AL.mult, AL.add)
                    # q2 = vv * fi (gpsimd, in place on vv)
                    nc.gpsimd.tensor_tensor(out=vv[:], in0=vv[:],
                                            in1=fi_t[:, ql], op=AL.mult)
                    # q1 = u * fv (in place on t_q); out = q1 + q2
                    nc.vector.tensor_tensor(out=t_q[:], in0=t_q[:],
                                            in1=fv_t[:, ql], op=AL.mult)
                    nc.vector.tensor_tensor(out=t_q[:], in0=t_q[:],
                                            in1=vv[:], op=AL.add)
                    nc.sync.dma_start(d_out[s][:, ql], t_q[:])

    nc.compile()
    return nc


def _host_consts(ca1_w, ca1_b, bn_a_g, bn_a_b, bn_a_m, bn_a_v,
                 ca2_w, ca2_b, bn_b_g, bn_b_b, bn_b_m, bn_b_v,
                 conv1_w, conv1_b, bn_c_g, bn_c_b, bn_c_m, bn_c_v,
                 chatt_w1, chatt_w2, sa_w):
    bf = ml_dtypes.bfloat16
    f = np.float32
    k_a = bn_a_g / np.sqrt(bn_a_v + EPS)
    w1 = ca1_w * k_a[:, None]
    b1 = (ca1_b - bn_a_m) * k_a + bn_a_b
    k_b = bn_b_g / np.sqrt(bn_b_v + EPS)
    w2 = ca2_w * k_b[:, None]
    b2 = (ca2_b - bn_b_m) * k_b + bn_b_b
    s_c = bn_c_g / np.sqrt(bn_c_v + EPS)
    b_c = (conv1_b - bn_c_m) * s_c + bn_c_b
    # BN scale folded into the conv weights (scales output channel o)
    wvi_t = np.ascontiguousarray(conv1_w[:, :C].T) * s_c[None, :]
    wir_t = np.ascontiguousarray(conv1_w[:, C:].T) * s_c[None, :]
    bmat = np.zeros((14, 128, 128), np.float32)
    for chn in range(2):
        scale = (1.0 / 128.0) if chn == 0 else 1.0
        for dy in range(7):
            for dx in range(7):
                off = dx - 3          # w' - w
                v = sa_w[0, chn, dy, dx] * scale
                if off >= 0:
                    idx = np.arange(0, 128 - off)
                    bmat[chn * 7 + dy, idx + off, idx] = v
                else:
                    idx = np.arange(-off, 128)
                    bmat[chn * 7 + dy, idx + off, idx] = v
    return {
        "wvi_t": wvi_t.astype(bf),
        "wir_t": wir_t.astype(bf),
        "bmat": bmat.astype(bf),
        "l1v": np.ascontiguousarray((w1[:, :C] / P).T).astype(f),
        "l1i": np.ascontiguousarray((w1[:, C:] / P).T).astype(f),
        "b1": b1.reshape(-1, 1).astype(f),
        "l2": np.ascontiguousarray(w2.T).astype(f),
        "b2": b2.reshape(-1, 1).astype(f),
        "bc": b_c.reshape(-1, 1).astype(f),
        "c1a": np.ascontiguousarray((chatt_w1 / P).T).astype(f),
        "c1m": np.ascontiguousarray(chatt_w1.T).astype(f),
        "c2r": np.ascontiguousarray(chatt_w2.T).astype(f),
        "eye": np.eye(C, dtype=f).astype(bf),
    }


def kernel(f_vi, f_ir, ca1_w, ca1_b, bn_a_g, bn_a_b, bn_a_m, bn_a_v,
           ca2_w, ca2_b, bn_b_g, bn_b_b, bn_b_m, bn_b_v,
           conv1_w, conv1_b, bn_c_g, bn_c_b, bn_c_m, bn_c_v,
           chatt_w1, chatt_w2, sa_w, _trace=False):
    if "nc" not in _cache:
        _cache["nc"] = _build_program()
    nc = _cache["nc"]

    consts = _host_consts(
        np.asarray(ca1_w, np.float32), np.asarray(ca1_b, np.float32),
        np.asarray(bn_a_g, np.float32), np.asarray(bn_a_b, np.float32),
        np.asarray(bn_a_m, np.float32), np.asarray(bn_a_v, np.float32),
        np.asarray(ca2_w, np.float32), np.asarray(ca2_b, np.float32),
        np.asarray(bn_b_g, np.float32), np.asarray(bn_b_b, np.float32),
        np.asarray(bn_b_m, np.float32), np.asarray(bn_b_v, np.float32),
        np.asarray(conv1_w, np.float32), np.asarray(conv1_b, np.float32),
        np.asarray(bn_c_g, np.float32), np.asarray(bn_c_b, np.float32),
        np.asarray(bn_c_m, np.float32), np.asarray(bn_c_v, np.float32),
        np.asarray(chatt_w1, np.float32), np.asarray(chatt_w2, np.float32),
        np.asarray(sa_w, np.float32))

    bf = ml_dtypes.bfloat16
    fv = np.asarray(f_vi, np.float32).reshape(N, C, P).astype(bf)
    fi = np.asarray(f_ir, np.float32).reshape(N, C, P).astype(bf)
    in_maps = []
    for i in range(NCORES):
        m = dict(consts)
        m["f_vi"] = np.ascontiguousarray(fv[i * SPC:(i + 1) * SPC])
        m["f_ir"] = np.ascontiguousarray(fi[i * SPC:(i + 1) * SPC])
        in_maps.append(m)

    res = bass_utils.run_bass_kernel_spmd(nc, in_maps, core_ids=list(range(NCORES)),
                                          trace=_trace)
    if _trace:
        _cache["last_trace"] = res
    out = np.concatenate([res.results[i]["out"] for i in range(NCORES)], axis=0)
    return out.astype(np.float32).reshape(N, C, H, W)
